# revision 32
# baseline (speedup 1.0000x reference)
"""GTA3Layer Trainium2 kernel (v5: v3 attention + restructured tail).

Sharding: 8 cores = 2 batches x 4 query-blocks of 512 rows. Each core
computes its 512 output rows end-to-end (attention over all 8 heads +
the live key range, then Wo/residual/LN/FFN/LN); no collectives.

v5 over v3 (sim 108.3 -> 104.0 us; v3 measured 125 us HW; v5 output
HW-verified bit-identical error to v3, rel err 4.85e-4):
  - normalize: per-strip rowsum rows gathered into one [4,QB] tile,
    ONE batched add/reciprocal/mask chain (replaces 4 per-strip
    reciprocal chains per group, ~24 tiny DVE ops -> 6).
  - tail split into two query-halves whose emission is zip-interleaved
    (generators with yields): the in-order engine queues otherwise
    serialize the halves completely. Accumulation groups (Wo 8-matmul,
    W2 pair) stay atomic within a half.
  - LN x^2 and mu^2 on Act (Square lives in both act-table sets), FFN
    relu on Act (Relu is in the sqrt set) -> tail DVE work ~halved;
    a dummy [1,1] Sqrt right after attention prefetches the exp->sqrt
    table switch off the LN critical path.
  - per-half output DMA.
All attention/projection code and DMA layout are byte-identical to v3
(the v4 attention restructure - rank-1 PE biases, 32-wide value slots,
group-outer loop, packed input tensors, sim 91 us - faults the device
at runtime; see kernel_v4_wip.py and the memory notes for the bisect
state). Drip-feeding group-0's normalize into group-1's attention was
tried and is sim-neutral (104.0 either way) - not worth the risk.

Hardware constraints (HW-verified by bisection in v3; violating any
one crashes the device at runtime even though compile + sim pass):
  - compute-engine APs must start at 32-aligned partitions; same for
    DMA partition bases (single-row SBUF reads at 32s+16 are proven).
  - 16-bit matmuls: no row-packed tile_position=(32s,0); column
    packing (0,32s) is fine.
  - never read PSUM regions no matmul has written.
  - no novel PE matmuls between another accumulation group's
    start/stop (only the baseline logit/att@v interleave survives).
  - work tile-pool ring depth capped at 3 (bufs=5/8 fault the device).
  - Rsqrt/Reciprocal/Ln activations unusable (framework block / table
    fixpoint pins Exp to set 0 and would thrash on Ln).

Math notes (vs the jax reference):
  - softmax(s/4) then phi (pow(alpha,A) mask + L1 renorm) fuse: the
    softmax denominator cancels, so att = f*exp(s/4)/sum_m(...) with
    f = pow(alpha+1e-10, A)*(A != 0); logits O(1) so no max-subtract.
  - key-padding folded into V and the ones-column, query-padding into
    the per-row reciprocal; k-projection bias cancels -> dropped.
  - row-sums ride the att@v matmul as a 17th "ones" column of V.

Layout: transposed ([d,n]/[m,q]) so the PE contracts over partitions
and LN stats come from ones-matmuls; per-head 16-dim slices on
32-partition strips so four heads' K=16 matmuls pack into the PE via
tile_position. Logit/projection matmuls f32r; att weights and V f16.
"""

import numpy as np
from contextlib import ExitStack

B, N, D, H, DH = 2, 2048, 128, 8, 16
NSPLIT = 1
NCORES = 8
QB = 512          # query rows per core


def _stripe_cols(W):
    outs = []
    for g in range(2):
        o = np.zeros((D, D), np.float32)
        for s in range(4):
            h = 4 * g + s
            o[:, 32 * s:32 * s + 16] = W[:, 16 * h:16 * h + 16]
        outs.append(o)
    return outs


def _stripe_rows(W):
    outs = []
    for g in range(2):
        o = np.zeros((D, D), np.float32)
        for s in range(4):
            h = 4 * g + s
            o[32 * s:32 * s + 16, :] = W[16 * h:16 * h + 16, :]
        outs.append(o)
    return outs


def _stripe_vec(b):
    outs = []
    for g in range(2):
        o = np.zeros((D, 1), np.float32)
        for s in range(4):
            h = 4 * g + s
            o[32 * s:32 * s + 16, 0] = b[16 * h:16 * h + 16]
        outs.append(o)
    return outs


def _build_program(NT, repeat=1):
    import concourse.bacc as bacc
    import concourse.tile as tile
    import concourse.mybir as mybir

    f32 = mybir.dt.float32
    f32r = mybir.dt.float32r
    f16 = mybir.dt.float16
    AF = mybir.ActivationFunctionType
    OP = mybir.AluOpType

    nc = bacc.Bacc(trn_type="TRN2")

    def din(name, shape, dtype=f32):
        return nc.dram_tensor(name, shape, dtype, kind="ExternalInput")

    NK = NT * 128
    NPAIR = (NT + 1) // 2
    NCH = (NK + 511) // 512
    hT_d = din("hT", [D, NK], f32r)
    hTq_d = din("hTq", [D, QB], f32r)
    fTd_d = din("fTd", [128, NT * QB], f16)
    WqA_d = din("WqA", [D, D], f32r); WqB_d = din("WqB", [D, D], f32r)
    bqA_d = din("bqA", [D, 1]); bqB_d = din("bqB", [D, 1])
    WkA_d = din("WkA", [D, D], f32r); WkB_d = din("WkB", [D, D], f32r)
    hTv_d = din("hTv", [D, NT * 128], f16)
    Wv_d = din("Wv", [D, D], f16)
    bvk_d = din("bvk", [128, NT, D])
    kmm_d = din("kmm", [128, NT])
    qm4_d = din("qm4", [4, QB])
    onesrow_d = din("onesrow", [1, D], f32r)
    WoA_d = din("WoA", [D, D], f32r); WoB_d = din("WoB", [D, D], f32r)
    boc_d = din("boc", [D, 1])
    W1_d = din("W1", [D, 2 * D], f32r)
    b1c_d = din("b1c", [D, 2])
    W2a_d = din("W2a", [D, D], f32r); W2b_d = din("W2b", [D, D], f32r)
    b2c_d = din("b2c", [D, 1])
    g1r_d = din("g1r", [1, D], f32r)
    g2r_d = din("g2r", [1, D], f32r)
    be2c_d = din("be2c", [D, 1])
    xout_d = nc.dram_tensor("xout", [D, QB], f32, kind="ExternalOutput")

    with tile.TileContext(nc) as tc, ExitStack() as ctx:
        const = ctx.enter_context(tc.tile_pool(name="const", bufs=1))
        big = ctx.enter_context(tc.tile_pool(name="big", bufs=1))
        work = ctx.enter_context(tc.tile_pool(name="work", bufs=3))
        rows = ctx.enter_context(tc.tile_pool(name="rows", bufs=1))
        ps_s = ctx.enter_context(tc.tile_pool(name="ps_s", bufs=2, space="PSUM"))
        ps_o = ctx.enter_context(tc.tile_pool(name="ps_o", bufs=2, space="PSUM"))
        ps_m = ctx.enter_context(tc.tile_pool(name="ps_m", bufs=2, space="PSUM"))

        def load(pool, dram, tag):
            t = pool.tile(list(dram.shape), dram.dtype, tag=tag)
            nc.sync.dma_start(out=t[:], in_=dram[:])
            return t

        Wq = [load(const, WqA_d, "WqA"), load(const, WqB_d, "WqB")]
        bq = [load(const, bqA_d, "bqA"), load(const, bqB_d, "bqB")]
        hTq = load(big, hTq_d, "hTq")
        Wk = [load(const, WkA_d, "WkA"), load(const, WkB_d, "WkB")]
        hT = load(big, hT_d, "hT")
        hTv = load(big, hTv_d, "hTv")
        Wv = load(const, Wv_d, "Wv")
        bvk = load(const, bvk_d, "bvk")
        kmm = load(const, kmm_d, "kmm")
        fT = load(big, fTd_d, "fT")
        qm4 = load(const, qm4_d, "qm4")
        onesrow = load(const, onesrow_d, "onesrow")
        bc0 = big.tile([128, QB], f32, tag="bc0")
        bc1 = big.tile([128, QB], f32, tag="bc1")
        bcl = [bc0, bc1]
        Wo = [load(const, WoA_d, "WoA"), load(const, WoB_d, "WoB")]
        boc = load(const, boc_d, "boc")
        W1 = load(const, W1_d, "W1")
        b1c = load(const, b1c_d, "b1c")
        W2a = load(const, W2a_d, "W2a")
        W2b = load(const, W2b_d, "W2b")
        b2c = load(const, b2c_d, "b2c")
        g1r = load(const, g1r_d, "g1r")
        g2r = load(const, g2r_d, "g2r")
        be2c = load(const, be2c_d, "be2c")

        epsr = const.tile([1, QB], f32, tag="epsr")
        nc.vector.memset(epsr[:], 1e-30)
        ones = const.tile([128, 1], f32r, tag="ones")
        nc.vector.memset(ones[:].bitcast(f32), 1.0)
        eps = const.tile([1, 1], f32, tag="eps")
        nc.vector.memset(eps[:], 1e-5)

        for _rep in range(repeat):
            # ---- q/k/v projections (transposed layouts) ----
            qT = []
            for g in range(2):
                p = ps_m.tile([128, QB], f32, tag="misc")
                nc.tensor.matmul(p[:], lhsT=Wq[g][:], rhs=hTq[:],
                                 start=True, stop=True)
                t = big.tile([128, QB], f32r, tag=f"qT{g}")
                nc.scalar.activation(t[:], p[:], AF.Identity, bias=bq[g][:, 0:1])
                qT.append(t)

            kT = []
            for g in range(2):
                t = big.tile([128, NK], f32r, tag=f"kT{g}")
                for u in range(NCH):
                    wdt = min(512, NK - 512 * u)
                    p = ps_m.tile([128, 512], f32, tag="misc")
                    nc.tensor.matmul(
                        p[:, :wdt], lhsT=Wk[g][:],
                        rhs=hT[:, 512 * u:512 * u + wdt],
                        start=True, stop=True)
                    nc.vector.tensor_copy(t[:, 512 * u:512 * u + wdt],
                                          p[:, :wdt])
                kT.append(t)

            # v in [key, d] layout, per-head 17-wide slots (16 dims + ones
            # col = key mask), fp16
            vS = big.tile([128, NT, H, 17], f16, tag="vS")
            for j in range(NT):
                p = ps_m.tile([128, D], f32, tag="misc")
                nc.tensor.matmul(
                    p[:], lhsT=hTv[:, 128 * j:128 * (j + 1)], rhs=Wv[:],
                    start=True, stop=True)
                nc.vector.scalar_tensor_tensor(
                    out=vS[:, j, :, 0:16],
                    in0=p.rearrange("p (h e) -> p h e", h=H),
                    scalar=kmm[:, j:j + 1],
                    in1=bvk[:, j, :].rearrange("p (h e) -> p h e", h=H),
                    op0=OP.mult, op1=OP.add)
            # all NT ones-columns (key mask) in one strided copy
            nc.vector.tensor_copy(
                vS[:, :, :, 16:17],
                kmm[:, :, None, None].to_broadcast([128, NT, H, 1]))

            # ---- attention: j-pairs, 4-head strip packing. Both groups'
            # attention is emitted before either normalize so DVE's
            # in-order queue never interleaves normalize row-math with
            # the wm multiplies that feed the exp work-ring. ----
            oT = []
            psos = []
            for g in range(2):
                pso = ps_o.tile([128, QB], f32, tag="pso")
                psos.append(pso)
                for jp in range(NPAIR):
                    jlist = [j for j in (2 * jp, 2 * jp + 1) if j < NT]
                    wdt = 512 * len(jlist)
                    for s in range(4):
                        pss = ps_s.tile([128, 1024], f32, tag="pss")
                        for u, j in enumerate(jlist):
                            nc.tensor.matmul(
                                pss[:, 512 * u:512 * (u + 1)],
                                lhsT=kT[g][32 * s:32 * s + 16,
                                           128 * j:128 * (j + 1)],
                                rhs=qT[g][32 * s:32 * s + 16, :],
                                start=True, stop=True,
                                tile_position=(32 * s, 0))
                        w = work.tile([128, 1024], f16, tag="w")
                        nc.scalar.activation(w[:, :wdt], pss[:, :wdt],
                                             AF.Exp, scale=0.25)
                        wm = work.tile([128, 1024], f16, tag="wm")
                        nc.vector.tensor_mul(
                            wm[:, :wdt], w[:, :wdt],
                            fT[:, 1024 * jp:1024 * jp + wdt])
                        for u, j in enumerate(jlist):
                            nc.tensor.matmul(
                                pso[32 * s:32 * s + 17, :],
                                lhsT=vS[:, j, 4 * g + s, :],
                                rhs=wm[:, 512 * u:512 * (u + 1)],
                                start=(j == 0), stop=(j == NT - 1),
                                tile_position=(0, 32 * s),
                                skip_group_check=True)

            # prefetch the sqrt-set act-table switch while DVE does
            # the normalize row math (all exps are done by here).
            dum = rows.tile([1, 1], f32, tag="dum")
            nc.scalar.activation(dum[:], eps[:], AF.Sqrt)
            for g in range(2):
                # normalize: per-strip copies (v3 pattern), then batched
                # [4,QB] reciprocal chain and ONE full-tile multiply.
                pso = psos[g]
                t128 = work.tile([128, QB], f32, tag="t128")
                for s in range(4):
                    nc.vector.tensor_copy(t128[32 * s:32 * s + 17, :],
                                          pso[32 * s:32 * s + 17, :])
                r4 = rows.tile([4, QB], f32, tag=f"r4{g}", name=f"r4{g}")
                for s in range(4):
                    nc.sync.dma_start(out=r4[s:s + 1, :],
                                      in_=t128[32 * s + 16:32 * s + 17, :])
                ra = rows.tile([4, QB], f32, tag=f"ra{g}", name=f"ra{g}")
                nc.vector.tensor_scalar_add(ra[:], r4[:], 1e-30)
                rr = rows.tile([4, QB], f32, tag=f"rr{g}", name=f"rr{g}")
                nc.vector.reciprocal(rr[:], ra[:])
                rq = rows.tile([4, QB], f32, tag=f"rq{g}", name=f"rq{g}")
                nc.vector.tensor_mul(rq[:], rr[:], qm4[:])
                for s in range(4):
                    nc.sync.dma_start(
                        out=bcl[g][32 * s:32 * s + 16, :],
                        in_=rq[s:s + 1, None, :].to_broadcast([1, 16, QB]))
                ot = big.tile([128, QB], f32r, tag=f"oT{g}", name=f"oT{g}")
                for s in range(4):
                    nc.vector.tensor_mul(
                        ot[32 * s:32 * s + 16, :],
                        t128[32 * s:32 * s + 16, :],
                        bcl[g][32 * s:32 * s + 16, :])
                oT.append(ot)

            # ---- tail: Wo + residual + LN1 + FFN + LN2, two query-
            # halves zip-interleaved into the in-order engine queues ----
            QH = QB // 2

            def layernorm(xpair, grow, becol, out_ap, h):
                # xpair [128, 2, QH]: [:,0]=x, [:,1]=x^2 (Act Square).
                # mu^2 via Act Square(sr/D) in parallel with mu on DVE.
                nc.scalar.activation(xpair[:, 1, :], xpair[:, 0, :],
                                     AF.Square)
                sr = ps_m.tile([128, QB], f32, tag="misc", name=f"sr{h}")
                nc.tensor.matmul(sr[0:1, 0:QH], lhsT=ones[:],
                                 rhs=xpair[:, 0, :], start=True, stop=True)
                yield
                qr = ps_m.tile([128, QB], f32, tag="misc", name=f"qr{h}")
                nc.tensor.matmul(qr[0:1, 0:QH], lhsT=ones[:],
                                 rhs=xpair[:, 1, :], start=True, stop=True)
                mu = rows.tile([1, QH], f32r, tag=f"m1{h}", name=f"m1{h}")
                with nc.allow_low_precision(reason="f32r is full fp32"):
                    nc.vector.tensor_scalar_mul(mu[:], sr[0:1, 0:QH], 1.0 / D)
                musq = rows.tile([1, QH], f32, tag=f"m2{h}", name=f"m2{h}")
                nc.scalar.activation(musq[:], sr[0:1, 0:QH], AF.Square,
                                     scale=1.0 / D)
                yield
                var = rows.tile([1, QH], f32, tag=f"m3{h}", name=f"m3{h}")
                nc.vector.scalar_tensor_tensor(
                    out=var[:], in0=qr[0:1, 0:QH], scalar=1.0 / D,
                    in1=musq[:], op0=OP.mult, op1=OP.subtract)
                sd = rows.tile([1, QH], f32, tag=f"m4{h}", name=f"m4{h}")
                nc.scalar.activation(sd[:], var[:], AF.Sqrt, bias=eps[:, 0:1])
                yield
                rstd = rows.tile([1, QH], f32r, tag=f"m5{h}", name=f"m5{h}")
                with nc.allow_low_precision(reason="f32r is full fp32"):
                    nc.vector.reciprocal(rstd[:], sd[:])
                mub = ps_m.tile([128, QB], f32, tag="misc", name=f"mub{h}")
                nc.tensor.matmul(mub[:, 0:QH], lhsT=onesrow[:], rhs=mu[:],
                                 start=True, stop=True)
                rsb = ps_m.tile([128, QB], f32, tag="misc", name=f"rsb{h}")
                nc.tensor.matmul(rsb[:, 0:QH], lhsT=grow[:], rhs=rstd[:],
                                 start=True, stop=True)
                yield
                t1 = work.tile([128, QH], f32, tag=f"t1{h}", name=f"t1{h}")
                nc.vector.tensor_sub(t1[:], xpair[:, 0, :], mub[:, 0:QH])
                yield
                if becol is None:
                    with nc.allow_low_precision(reason="f32r is full fp32"):
                        nc.vector.tensor_mul(out_ap, t1[:], rsb[:, 0:QH])
                else:
                    nc.vector.tensor_mul(t1[:], t1[:], rsb[:, 0:QH])
                    nc.scalar.activation(out_ap, t1[:], AF.Identity,
                                         bias=becol[:, 0:1])
                yield

            def tail_half(hh):
                # yields between ops so the two halves interleave in the
                # in-order engine queues (emitted back-to-back they would
                # serialize); accumulation groups stay atomic.
                qs = slice(QH * hh, QH * (hh + 1))
                psy = ps_m.tile([128, QB], f32, tag="misc", name=f"psy{hh}")
                k8 = 0
                for g in range(2):
                    for s in range(4):
                        nc.tensor.matmul(
                            psy[:, 0:QH],
                            lhsT=Wo[g][32 * s:32 * s + 16, :],
                            rhs=oT[g][32 * s:32 * s + 16, qs],
                            start=(k8 == 0), stop=(k8 == 7),
                            tile_position=(32 * s, 0))
                        k8 += 1
                yield
                xx1 = big.tile([128, 2, QH], f32r, tag=f"xx1{hh}")
                nc.vector.scalar_tensor_tensor(
                    out=xx1[:, 0, :], in0=psy[:, 0:QH], scalar=boc[:, 0:1],
                    in1=hTq[:, qs], op0=OP.add, op1=OP.add)
                yield
                x2 = big.tile([128, QH], f32r, tag=f"x2{hh}")
                yield from layernorm(xx1, g1r, None, x2[:], 2 * hh)
                f1 = []
                for u in range(2):
                    p = ps_m.tile([128, QB], f32, tag="misc",
                                  name=f"f1p{u}{hh}")
                    nc.tensor.matmul(p[:, 0:QH],
                                     lhsT=W1[:, 128 * u:128 * (u + 1)],
                                     rhs=x2[:], start=True, stop=True)
                    t = big.tile([128, QH], f32r, tag=f"f1{u}{hh}",
                                 name=f"f1{u}{hh}")
                    nc.scalar.activation(t[:], p[:, 0:QH], AF.Relu,
                                         bias=b1c[:, u:u + 1])
                    f1.append(t)
                    yield
                psy2 = ps_m.tile([128, QB], f32, tag="misc",
                                 name=f"psy2{hh}")
                nc.tensor.matmul(psy2[:, 0:QH], lhsT=W2a[:], rhs=f1[0][:],
                                 start=True, stop=False)
                nc.tensor.matmul(psy2[:, 0:QH], lhsT=W2b[:], rhs=f1[1][:],
                                 start=False, stop=True)
                yield
                xx2 = big.tile([128, 2, QH], f32r, tag=f"xx2{hh}")
                nc.vector.scalar_tensor_tensor(
                    out=xx2[:, 0, :], in0=psy2[:, 0:QH], scalar=b2c[:, 0:1],
                    in1=x2[:], op0=OP.add, op1=OP.add)
                yield
                xf = big.tile([128, QH], f32, tag=f"xf{hh}")
                yield from layernorm(xx2, g2r, be2c, xf[:], 2 * hh + 1)
                nc.sync.dma_start(out=xout_d[:, qs], in_=xf[:])

            gens = [tail_half(0), tail_half(1)]
            live = [True, True]
            while any(live):
                for i in range(2):
                    if live[i]:
                        live[i] = next(gens[i], "end") != "end"

    return nc


def _host_inputs(h, A, lengths, alpha, Wq, bq, Wk, bk, Wv, bv, Wo, bo,
                 W1, b1, W2, b2, g1, be1, g2, be2):
    """Build the 8 per-core input maps."""
    h = np.asarray(h, np.float32)
    A = np.asarray(A)
    lengths = np.asarray(lengths)
    al = max(float(np.float32(alpha)), 0.0)
    NT = min(max(1, int(np.ceil(int(lengths.max()) / 128.0))), N // 128)
    NK = NT * 128
    amax = int(A.max())
    powtab = np.power(np.float32(al) + np.float32(1e-10),
                      np.arange(amax + 1, dtype=np.float32))
    powtab[0] = 0.0
    fTfull = powtab[A]

    WqS = _stripe_cols(np.asarray(Wq, np.float32))
    bqS = _stripe_vec(np.asarray(bq, np.float32))
    WkS = _stripe_cols(np.asarray(Wk, np.float32))
    WoS = _stripe_rows(np.asarray(Wo, np.float32))
    Wv32 = np.ascontiguousarray(np.asarray(Wv, np.float32).astype(np.float16))
    W1a = np.ascontiguousarray(np.asarray(W1, np.float32))
    W2_ = np.asarray(W2, np.float32)
    b1_ = np.asarray(b1, np.float32)

    onesrow = np.ones((1, D), np.float32)
    common = dict(
        onesrow=onesrow,
        WqA=WqS[0], WqB=WqS[1], bqA=bqS[0], bqB=bqS[1],
        WkA=WkS[0], WkB=WkS[1], Wv=Wv32,
        WoA=np.ascontiguousarray(WoS[0]), WoB=np.ascontiguousarray(WoS[1]),
        boc=np.asarray(bo, np.float32).reshape(D, 1).copy(),
        W1=W1a,
        b1c=np.ascontiguousarray(
            (b1_ + np.asarray(be1, np.float32) @ W1a).reshape(2, D).T),
        W2a=np.ascontiguousarray(W2_[:D]), W2b=np.ascontiguousarray(W2_[D:]),
        b2c=np.ascontiguousarray(
            (np.asarray(b2, np.float32)
             + np.asarray(be1, np.float32)).reshape(D, 1)),
        g1r=np.asarray(g1, np.float32).reshape(1, D).copy(),
        g2r=np.asarray(g2, np.float32).reshape(1, D).copy(),
        be2c=np.asarray(be2, np.float32).reshape(D, 1).copy(),
    )

    in_maps = []
    for c in range(NCORES):
        b = c // 4
        q0 = (c % 4) * QB
        L = int(lengths[b])
        kmask = (np.arange(NK) < L).astype(np.float32)
        m = dict(common)
        m["hT"] = np.ascontiguousarray(h[b].T[:, :NK])
        m["hTv"] = m["hT"].astype(np.float16)
        m["hTq"] = np.ascontiguousarray(h[b, q0:q0 + QB].T)
        ft = fTfull[b, q0:q0 + QB, :NK].astype(np.float16)
        ft = ft.T.reshape(NT, 128, QB).transpose(1, 0, 2)
        m["fTd"] = np.ascontiguousarray(ft.reshape(128, NT * QB))
        m["bvk"] = np.ascontiguousarray(
            (np.asarray(bv, np.float32)[None, :] * kmask[:, None])
            .reshape(NT, 128, D).transpose(1, 0, 2))
        m["kmm"] = np.ascontiguousarray(kmask.reshape(NT, 128).T)
        qmrow = (np.arange(q0, q0 + QB) < L).astype(np.float32)
        m["qm4"] = np.ascontiguousarray(np.tile(qmrow, (4, 1)))
        in_maps.append(m)
    return NT, in_maps


_CACHE = {}
TRACE = False


def kernel(**inputs):
    import os
    from concourse.bass_utils import run_bass_kernel_spmd

    # The NTFF trace path needs antenv.axon_hooks, absent in this
    # container — make sure an inherited BASS_TRACE can't select it.
    os.environ["BASS_NEVER_TRACE"] = "1"
    NT, in_maps = _host_inputs(**inputs)
    if ("nc", NT) not in _CACHE:
        nc = _build_program(NT)
        nc.finalize()
        _CACHE[("nc", NT)] = nc
    nc = _CACHE[("nc", NT)]
    res = run_bass_kernel_spmd(nc, in_maps, core_ids=list(range(NCORES)),
                               trace=TRACE)
    _CACHE["last"] = res
    out = np.empty((B, N, D), np.float32)
    for c in range(NCORES):
        b = c // 4
        q0 = (c % 4) * QB
        out[b, q0:q0 + QB, :] = res.results[c]["xout"].T
    return out

